# revision 27
# baseline (speedup 1.0000x reference)
"""Channel-attention kernel for Trainium2 (8 NeuronCores, SPMD).

Reference computation (B=2, C=512, H=W=64, heads=8, hd=64, N=H*W=4096):
    tokens = x.transpose(0,2,3,1).reshape(B,N,C)
    qkv    = tokens @ w_qkv.T -> q,k,v per head    (k scaled by hd**-0.5)
    attn   = softmax(k @ v.T, axis=-1)             # [B,h,N,N]
    out    = attn @ q                              # [B,h,N,hd]
    out -> (B,N,h,hd) -> (B,H,W,C) -> (B,C,H,W) -> reshape (B,N,C)
    y      = out @ w_proj.T + b_proj -> reshape (B,C,H,W)

Sharding: 16 (batch, head) pairs over 8 cores -> each core handles one
batch element and two adjacent heads (head-separable end to end, incl.
the projection, thanks to the raw (B,C,H,W)->(B,N,C) reinterpretation).

Per-core pipeline (v2, rebuilt around measured engine rooflines):
  * S^T = (K V^T)^T per (head, m-block, n-chunk) via ONE fp8e4 DoubleRow
    matmul: k-tile slot 0 = (k8, v8), slot 1 = (dk8, v8) where dk8 is the
    fp8 residual of k (k quantization dominates the attention error, so
    the "spare" DoubleRow slot carries its first-order correction).
    256 PE cycles per [128,512] tile instead of 512 (fp32r).
  * exp() is the machine bottleneck (ACT does 1 elem/lane/cycle, no fast
    mode).  Tiles are split between ACT (true exp, fused *hd^-0.5 scale)
    and DVE (Schraudolph bf16: i16 = round(A*S+B) bitcast to bf16, which
    tensor_scalar emits in a single pass).  GPSIMD cannot touch PSUM so
    it only mirrors the vTz slot copy.
  * O^T accumulates over m in PSUM with bf16 operands (E, q); the 65th
    lhsT column of ones accumulates the softmax denominator Z for free.
  * Per-n-chunk O^T is copied to SBUF (ACT), PE-transposed in 128-column
    strips, scaled by 1/Z (reciprocal on DVE, Copy*scale on ACT) into the
    proj-ready M^T layout, then Y = M @ w_proj.T + b_proj streams out.
  * Q is computed channel-major (Q^T) with 512-wide f32r matmuls and
    PE-transposed to token-major: f32r matmuls with <256 output columns
    run 4x slower (moving-operand fetch bound), so the naive token-major
    Q matmul is a trap.
  * PSUM budget (8 banks): 5-bank S ring (manually slotted so exp can
    batch adjacent pairs), 2-bank O accumulator, 1 bank for transposes.
  * x is DMA'd in 64 256-column pieces, quarter-major, so the first
    QKV matmuls start ~6us in instead of waiting for the full 8MB.
"""

import numpy as np

import concourse.bass as bass
import concourse.mybir as mybir
import concourse.tile as tile
from concourse import bacc, bass_utils
from concourse.bass import ts
from concourse.masks import make_identity

F32 = mybir.dt.float32
F32R = mybir.dt.float32r
BF16 = mybir.dt.bfloat16
FP8 = mybir.dt.float8e4
I16 = mybir.dt.int16
EXP = mybir.ActivationFunctionType.Exp
COPY = mybir.ActivationFunctionType.Copy
DR = mybir.MatmulPerfMode.DoubleRow
MULT = mybir.AluOpType.mult
ADD = mybir.AluOpType.add
SUB = mybir.AluOpType.subtract

B, C, H, W = 2, 512, 64, 64
N = H * W                 # 4096
HEADS_TOTAL = 8
HD = C // HEADS_TOTAL     # 64
SCALE = HD ** -0.5        # folded into exp(), NOT into wk (fp8 k stays full-range)
N_CORES = 8
HPC = 2                   # heads per core
NB = N // 128             # 32 m-blocks
NJ = N // 512             # 8 n-chunks
CC = C // 128             # 4 contraction chunks
LAG = 4                   # O-matmul pairs lag behind exp pairs
NSLOT = 5                 # S psum ring slots (banks)
# Schraudolph constants: bf16 bits of 2^(log2e*SCALE*S) ~= exp(S*SCALE)
A8 = 128.0 * 1.4426950408889634 * SCALE
B8 = 128.0 * (127.0 - 0.045)


def r(ap):
    """float32r view for plain-f32 PE operands (bit-identical, faster)."""
    return ap.bitcast(F32R) if ap.dtype == F32 else ap


def _emit(nc, tc):
    x_h = nc.dram_tensor("x", [C, N], F32R, kind="ExternalInput")
    wq_h = nc.dram_tensor("wq", [C, 128], F32R, kind="ExternalInput")
    wk_h = nc.dram_tensor("wk", [C, 128], F32R, kind="ExternalInput")
    wv_h = nc.dram_tensor("wv", [C, 128], F32R, kind="ExternalInput")
    wp_h = nc.dram_tensor("wp", [C, C], F32R, kind="ExternalInput")
    bp_h = nc.dram_tensor("bp", [1, C], F32, kind="ExternalInput")
    out_h = nc.dram_tensor("out", [HPC, 512, 512], F32, kind="ExternalOutput")

    singles = tc.alloc_tile_pool(name="singles", bufs=1)
    # pool-rotated PSUM tiles: separate memrefs per buffer — a single big
    # PSUM tile serializes S-matmuls behind the previous exp (per-memref
    # dependency state), locking the pipeline to ~2us per pair
    spool = tc.alloc_tile_pool(name="spool", bufs=5, space="PSUM")
    opool = tc.alloc_tile_pool(name="opool", bufs=2, space="PSUM")
    tpool = tc.alloc_tile_pool(name="tpool", bufs=1, space="PSUM")
    epool = tc.alloc_tile_pool(name="epool", bufs=6)
    vpool = tc.alloc_tile_pool(name="vpool", bufs=4)

    # ---- persistent SBUF tensors ----
    x_sb = singles.tile([128, CC, N], F32R)        # x[cc*128+p, n]
    wq_sb = singles.tile([128, CC, 128], F32R)
    wk_sb = singles.tile([128, CC, 128], F32R)
    wv_sb = singles.tile([128, CC, 128], F32R)
    wp_sb = singles.tile([128, CC, 512], F32R)
    bias_sb = singles.tile([128, 512], F32)
    id_sb = singles.tile([128, 128], F32)
    # DoubleRow operand layouts (partitions: head0 rows 0-63, head1 64-127)
    kTz = singles.tile([128, 2, N], FP8)           # slot0 = k8, slot1 = dk8
    vTz = singles.tile([128, 2, N], FP8)           # slot0 = slot1 = v8
    qTs = singles.tile([128, N], F32)              # Q^T channel-major staging
    qa = [singles.tile([128, NB, HD + 1], BF16, name=f"qa{h}") for h in range(HPC)]
    o_all = singles.tile([HD + 1, HPC, N], F32)    # O^T (+Z row) per head
    # f32r: feeds the proj matmul, which requires f32r-rounded producers
    mt = [singles.tile([128, CC, 512], F32R, name=f"mt{h}") for h in range(HPC)]



    make_identity(nc, id_sb)
    for h in range(HPC):
        nc.vector.memset(qa[h][:, :, HD:HD + 1], 1.0)

    # ---- input DMAs: small weights first, then x quarter-major ----
    nc.sync.dma_start(out=wq_sb, in_=wq_h.ap().rearrange("(cc p) m -> p cc m", p=128))
    nc.sync.dma_start(out=wk_sb, in_=wk_h.ap().rearrange("(cc p) m -> p cc m", p=128))
    nc.sync.dma_start(out=wv_sb, in_=wv_h.ap().rearrange("(cc p) m -> p cc m", p=128))
    x_view = x_h.ap().rearrange("(cc p) n -> p cc n", p=128)
    for q in range(4):
        for cc in range(CC):
            for piece in range(4):
                c0 = q * 1024 + piece * 256
                nc.sync.dma_start(
                    out=x_sb[:, cc, c0:c0 + 256], in_=x_view[:, cc, c0:c0 + 256]
                )
    nc.sync.dma_start(out=wp_sb, in_=wp_h.ap().rearrange("(cc p) m -> p cc m", p=128))
    nc.sync.dma_start(out=bias_sb, in_=bp_h.ap().to_broadcast((128, 512)))

    # ---- QKV phase ----
    def kv_group(w_sb, is_k, j8):
        kv_ps = spool.tile([128, 512], F32, tag="s", name="kv_ps")
        for cc in range(CC):
            nc.tensor.matmul(
                kv_ps,
                lhsT=r(w_sb[:, cc, :]),
                rhs=r(x_sb[:, cc, ts(j8, 512)]),
                start=(cc == 0),
                stop=(cc == CC - 1),
            )
        dz = kTz if is_k else vTz
        nc.vector.tensor_copy(out=dz[:, 0, ts(j8, 512)], in_=kv_ps)
        if is_k:  # slot1 = fp8 residual of k
            nc.vector.tensor_tensor(
                out=dz[:, 1, ts(j8, 512)], in0=kv_ps, in1=dz[:, 0, ts(j8, 512)], op=SUB
            )
        else:     # slot1 = copy of v8 (SBUF->SBUF, GPSIMD is otherwise idle)
            nc.gpsimd.tensor_copy(out=dz[:, 1, ts(j8, 512)], in_=dz[:, 0, ts(j8, 512)])

    def qT_group(j8):
        qt_ps = spool.tile([128, 512], F32, tag="s", name="qt_ps")
        for cc in range(CC):
            nc.tensor.matmul(
                qt_ps,
                lhsT=r(wq_sb[:, cc, :]),
                rhs=r(x_sb[:, cc, ts(j8, 512)]),
                start=(cc == 0),
                stop=(cc == CC - 1),
            )
        nc.vector.tensor_copy(out=qTs[:, ts(j8, 512)], in_=qt_ps)

    def qa_group(nb):
        # token-major q via PE transpose of qTs (f32r matmuls with <256
        # output columns are 4x slower, so q cannot be made token-major
        # directly at full speed)
        tq = tpool.tile([128, 128], F32, tag="t", name="tq")
        nc.tensor.transpose(tq, qTs[:, ts(nb, 128)], id_sb)
        for h in range(HPC):
            nc.scalar.activation(
                out=qa[h][:, nb, 0:HD], in_=tq[:, ts(h, HD)], func=COPY
            )

    # ---- attention: flat pair stream -------------------------------------
    # Pair p = (chunk j = p//NB, m-block i = p%NB).  Per slot the PE runs
    # [S(p)h0, S(p)h1, O(p-LAG)h0, O(p-LAG)h1]; exp(p) as two consecutive
    # half-tile instructions on ONE engine (ACT or DVE, alternating by a
    # ~18:14 ratio).  Single-bank S tiles rotate through a 5-deep ring, so
    # the S -> exp -> S-reuse WAR loop (~1.6us) spreads over 2.5 pairs and
    # stays off the critical path.  Keeping both halves of a pair on one
    # engine matters: a cross-engine pair makes every S wait on the more
    # backlogged of two queues.  Chunk boundaries dissolve into the flat
    # stream (the O chain and its drain trail LAG pairs behind).
    e_ring = {}
    o_ring = {}
    st = {"o_done": None, "tt": None}

    def emit_s_exp(p):
        j, i = p // NB, p % NB
        e_t = epool.tile([128, 2, 512], BF16, tag="e", name="e_t")
        for h in range(HPC):
            s_t = spool.tile([128, 512], F32, tag="s", name="s_t")
            nc.tensor.matmul(
                s_t,
                lhsT=vTz[ts(h, HD), :, ts(i, 128)],
                rhs=kTz[ts(h, HD), :, ts(j, 512)],
                start=True,
                stop=True,
                perf_mode=DR,
            )
            # h0 on ACT, h1 on DVE: the two halves run concurrently, and
            # each single-bank S tile's reuse WAR waits on only one engine
            if h == 0 or p % 32 == 0:
                nc.scalar.activation(
                    out=e_t[:, h, :], in_=s_t, func=EXP, scale=SCALE
                )
            else:
                nc.vector.tensor_scalar(
                    out=e_t.bitcast(I16)[:, h, :], in0=s_t,
                    scalar1=A8, scalar2=B8, op0=MULT, op1=ADD,
                )
        e_ring[p] = e_t

    def emit_drain(p):
        """Consumer-side work for pair p: O matmuls, then (trailing a
        finished chunk) the O->SBUF copies, transposes and normalization."""
        j, i = p // NB, p % NB
        if i == 0:
            o_ring[j] = [opool.tile([128, 512], F32, tag="o", name=f"o_ps{h}")
                         for h in range(HPC)]
        e_t = e_ring.pop(p)
        o_ps = o_ring[j]
        for h in range(HPC):
            nc.tensor.matmul(
                o_ps[h][0:HD + 1, :],
                lhsT=r(qa[h][:, i, :]),
                rhs=e_t[:, h, :],
                start=(i == 0),
                stop=(i == NB - 1),
            )
        if i == NB - 1:           # chunk j's O chain complete: copy out
            nc.scalar.activation(
                out=o_all[:, 0, ts(j, 512)], in_=o_ps[0][0:HD + 1, :],
                func=COPY,
            )
            nc.vector.tensor_copy(
                out=o_all[:, 1, ts(j, 512)], in_=o_ps[1][0:HD + 1, :]
            )
            st["o_done"] = j
            del o_ring[j]
        jd = st["o_done"]
        if jd is not None:
            # transposes of chunk jd trail the copies: h0 strips at slots
            # 2..5 (norm at 6), h1 reuses the same 4-slot tile at 7..10
            # (norm at 11) — one PSUM bank covers all 8 transposes
            if i in (2, 7):
                st["tt"] = tpool.tile([128, 4, HD + 1], F32, tag="t",
                                      name="tt")
            if 2 <= i <= 5 or 7 <= i <= 10:
                h, c = (0, i - 2) if i <= 5 else (1, i - 7)
                nc.tensor.transpose(
                    st["tt"][:, c, :], o_all[:, h, ts(jd * 4 + c, 128)],
                    id_sb[0:HD + 1, 0:HD + 1],
                )
            elif i in (6, 11):
                h = 0 if i == 6 else 1
                t_t = st["tt"]
                rz4 = vpool.tile([128, 4, 1], F32, tag="rz", name="rz4")
                nc.vector.reciprocal(out=rz4, in_=t_t[:, :, HD:HD + 1])
                for c in range(4):
                    nc.vector.tensor_scalar_mul(
                        mt[h][:, c, jd::8], t_t[:, c, 0:HD], rz4[:, c, :],
                    )

    # ---- QKV interleaved with attention chunk 0 (fills the x-DMA wait) ----
    for q4 in range(4):
        j0 = 2 * q4
        kv_group(wv_sb, False, j0)
        kv_group(wv_sb, False, j0 + 1)
        qT_group(j0)
        for nb in range(8 * q4, 8 * q4 + 4):
            qa_group(nb)
        kv_group(wk_sb, True, j0)
        kv_group(wk_sb, True, j0 + 1)
        qT_group(j0 + 1)
        for nb in range(8 * q4 + 4, 8 * q4 + 8):
            qa_group(nb)
        # chunk-0 pairs whose vT m-blocks this quarter just produced
        for p in range(8 * q4, 8 * q4 + 8):
            emit_s_exp(p)
            if p >= LAG:
                emit_drain(p - LAG)

    # ---- flat stream: chunks 1..7 + trailing drains ----
    NP = NJ * NB
    for p in range(NB, NP + LAG):
        if p < NP:
            emit_s_exp(p)
        emit_drain(p - LAG)

    # last chunk's transposes + norms (no following chunk to carry them)
    for h in range(HPC):
        t_t = tpool.tile([128, 4, HD + 1], F32, tag="t", name="tt")
        for c in range(4):
            nc.tensor.transpose(
                t_t[:, c, :], o_all[:, h, ts((NJ - 1) * 4 + c, 128)],
                id_sb[0:HD + 1, 0:HD + 1],
            )
        rz4 = vpool.tile([128, 4, 1], F32, tag="rz", name="rz4")
        nc.vector.reciprocal(out=rz4, in_=t_t[:, :, HD:HD + 1])
        for c in range(4):
            nc.vector.tensor_scalar_mul(
                mt[h][:, c, (NJ - 1)::8], t_t[:, c, 0:HD], rz4[:, c, :],
            )

    # ---- projection ----
    for h in range(HPC):
        for l in range(4):
            y_ps = spool.tile([128, 512], F32, tag="s", name="y_ps")
            for kk in range(CC):
                nc.tensor.matmul(
                    y_ps,
                    lhsT=r(mt[h][:, kk, ts(l, 128)]),
                    rhs=r(wp_sb[:, kk, :]),
                    start=(kk == 0),
                    stop=(kk == CC - 1),
                )
            y_sb = vpool.tile([128, 512], F32, tag="y", name="y_sb")
            nc.vector.tensor_add(out=y_sb, in0=y_ps, in1=bias_sb)
            nc.sync.dma_start(out=out_h.ap()[h, ts(l, 128), :], in_=y_sb)

    for pool in (vpool, epool, tpool, opool, spool, singles):
        pool.release()


_CACHE = {}


def _build():
    if "nc" not in _CACHE:
        nc = bacc.Bacc("TRN2", target_bir_lowering=False, debug=False)
        with tile.TileContext(nc) as tc:
            _emit(nc, tc)
        nc.compile()
        _CACHE["nc"] = nc
    return _CACHE["nc"]


def _shard(x, w_qkv, w_proj, b_proj):
    """Build the 8 per-core input maps from the full inputs."""
    wpT = np.ascontiguousarray(w_proj.T)
    bp = np.ascontiguousarray(b_proj.reshape(1, C))
    in_maps = []
    for core in range(N_CORES):
        b = core // 4
        h0 = HPC * (core % 4)
        r0 = h0 * HD
        in_maps.append({
            "x": np.ascontiguousarray(x[b].reshape(C, N)),
            "wq": np.ascontiguousarray(w_qkv[r0:r0 + 128, :].T),
            # NOTE: k left unscaled (hd**-0.5 folded into exp) so fp8
            # quantization sees full-range values
            "wk": np.ascontiguousarray(w_qkv[C + r0:C + r0 + 128, :].T),
            "wv": np.ascontiguousarray(w_qkv[2 * C + r0:2 * C + r0 + 128, :].T),
            "wp": wpT,
            "bp": bp,
        })
    return in_maps


def _gather(results):
    full = np.empty((B, C, N), dtype=np.float32)
    for core in range(N_CORES):
        b = core // 4
        h0 = HPC * (core % 4)
        y = results[core]["out"]  # [2, 512, 512]
        for hi in range(HPC):
            ch0 = (h0 + hi) * HD
            full[b, ch0:ch0 + HD] = y[hi].reshape(HD, N)
    return full.reshape(B, C, H, W)


def run(inputs, trace=False, **kw):
    nc = _build()
    in_maps = _shard(**inputs)
    res = bass_utils.run_bass_kernel_spmd(
        nc, in_maps, core_ids=list(range(N_CORES)), trace=trace, **kw
    )
    return _gather(res.results), res


def kernel(x, w_qkv, w_proj, b_proj):
    out, _ = run(dict(x=x, w_qkv=w_qkv, w_proj=w_proj, b_proj=b_proj))
    return out


# revision 33
# speedup vs baseline: 1.0607x; 1.0607x over previous
"""Channel-attention kernel for Trainium2 (8 NeuronCores, SPMD).

Reference computation (B=2, C=512, H=W=64, heads=8, hd=64, N=H*W=4096):
    tokens = x.transpose(0,2,3,1).reshape(B,N,C)
    qkv    = tokens @ w_qkv.T -> q,k,v per head    (k scaled by hd**-0.5)
    attn   = softmax(k @ v.T, axis=-1)             # [B,h,N,N]
    out    = attn @ q                              # [B,h,N,hd]
    out -> (B,N,h,hd) -> (B,H,W,C) -> (B,C,H,W) -> reshape (B,N,C)
    y      = out @ w_proj.T + b_proj -> reshape (B,C,H,W)

Sharding: 16 (batch, head) pairs over 8 cores -> each core handles one
batch element and two adjacent heads (head-separable end to end, incl.
the projection, thanks to the raw (B,C,H,W)->(B,N,C) reinterpretation).

Per-core pipeline (v2, rebuilt around measured engine rooflines):
  * S^T = (K V^T)^T per (head, m-block, n-chunk) via ONE fp8e4 DoubleRow
    matmul: k-tile slot 0 = (k8, v8), slot 1 = (dk8, v8) where dk8 is the
    fp8 residual of k (k quantization dominates the attention error, so
    the "spare" DoubleRow slot carries its first-order correction).
    256 PE cycles per [128,512] tile instead of 512 (fp32r).
  * exp() is the machine bottleneck (ACT does 1 elem/lane/cycle, no fast
    mode).  Tiles are split between ACT (true exp, fused *hd^-0.5 scale)
    and DVE (Schraudolph bf16: i16 = round(A*S+B) bitcast to bf16, which
    tensor_scalar emits in a single pass).  GPSIMD cannot touch PSUM so
    it only mirrors the vTz slot copy.
  * O^T accumulates over m in PSUM with bf16 operands (E, q); the 65th
    lhsT column of ones accumulates the softmax denominator Z for free.
  * Per-n-chunk O^T is copied to SBUF (ACT), PE-transposed in 128-column
    strips, scaled by 1/Z (reciprocal on DVE, Copy*scale on ACT) into the
    proj-ready M^T layout, then Y = M @ w_proj.T + b_proj streams out.
  * Q is computed channel-major (Q^T) with 512-wide f32r matmuls and
    PE-transposed to token-major: f32r matmuls with <256 output columns
    run 4x slower (moving-operand fetch bound), so the naive token-major
    Q matmul is a trap.
  * PSUM budget (8 banks): 5-bank S ring (manually slotted so exp can
    batch adjacent pairs), 2-bank O accumulator, 1 bank for transposes.
  * x is DMA'd in 64 256-column pieces, quarter-major, so the first
    QKV matmuls start ~6us in instead of waiting for the full 8MB.
"""

import numpy as np

import concourse.bass as bass
import concourse.mybir as mybir
import concourse.tile as tile
from concourse import bacc, bass_utils
from concourse.bass import ts
from concourse.masks import make_identity

F32 = mybir.dt.float32
F32R = mybir.dt.float32r
BF16 = mybir.dt.bfloat16
FP8 = mybir.dt.float8e4
I16 = mybir.dt.int16
EXP = mybir.ActivationFunctionType.Exp
COPY = mybir.ActivationFunctionType.Copy
DR = mybir.MatmulPerfMode.DoubleRow
MULT = mybir.AluOpType.mult
ADD = mybir.AluOpType.add
SUB = mybir.AluOpType.subtract

B, C, H, W = 2, 512, 64, 64
N = H * W                 # 4096
HEADS_TOTAL = 8
HD = C // HEADS_TOTAL     # 64
SCALE = HD ** -0.5        # folded into exp(), NOT into wk (fp8 k stays full-range)
N_CORES = 8
HPC = 2                   # heads per core
NB = N // 128             # 32 m-blocks
NJ = N // 512             # 8 n-chunks
CC = C // 128             # 4 contraction chunks
LAG = 4                   # O-matmul pairs lag behind exp pairs
NSLOT = 5                 # S psum ring slots (banks)
# Schraudolph constants: bf16 bits of 2^(log2e*SCALE*S) ~= exp(S*SCALE)
A8 = 128.0 * 1.4426950408889634 * SCALE
B8 = 128.0 * (127.0 - 0.045)


def r(ap):
    """float32r view for plain-f32 PE operands (bit-identical, faster)."""
    return ap.bitcast(F32R) if ap.dtype == F32 else ap


def _emit(nc, tc):
    x_h = nc.dram_tensor("x", [C, N], F32R, kind="ExternalInput")
    wq_h = nc.dram_tensor("wq", [C, 128], F32R, kind="ExternalInput")
    wk_h = nc.dram_tensor("wk", [C, 128], F32R, kind="ExternalInput")
    wv_h = nc.dram_tensor("wv", [C, 128], F32R, kind="ExternalInput")
    wp_h = nc.dram_tensor("wp", [C, C], F32R, kind="ExternalInput")
    bp_h = nc.dram_tensor("bp", [1, C], F32, kind="ExternalInput")
    out_h = nc.dram_tensor("out", [HPC, 512, 512], F32, kind="ExternalOutput")

    singles = tc.alloc_tile_pool(name="singles", bufs=1)
    # pool-rotated PSUM tiles: separate memrefs per buffer — a single big
    # PSUM tile serializes S-matmuls behind the previous exp (per-memref
    # dependency state), locking the pipeline to ~2us per pair
    # 6-bank S ring + 2-bank O accumulator = all 8 PSUM banks.  With 2 S
    # tiles per pair, an odd ring (5) gives a MINIMUM reuse distance of
    # 2.0 pairs (the even banks), which puts the ~1.6us S->exp->reuse WAR
    # loop on the critical path; 6 banks make it 3.0 pairs uniformly.
    # Transposes borrow the same ring (small tiles, occasional extra
    # allocations) instead of owning a bank.
    spool = tc.alloc_tile_pool(name="spool", bufs=6, space="PSUM")
    opool = tc.alloc_tile_pool(name="opool", bufs=2, space="PSUM")
    epool = tc.alloc_tile_pool(name="epool", bufs=6)
    vpool = tc.alloc_tile_pool(name="vpool", bufs=4)

    # ---- persistent SBUF tensors ----
    x_sb = singles.tile([128, CC, N], F32R)        # x[cc*128+p, n]
    wq_sb = singles.tile([128, CC, 128], F32R)
    wk_sb = singles.tile([128, CC, 128], F32R)
    wv_sb = singles.tile([128, CC, 128], F32R)
    wp_sb = singles.tile([128, CC, 512], F32R)
    bias_sb = singles.tile([128, 512], F32)
    id_sb = singles.tile([128, 128], F32)
    # DoubleRow operand layouts (partitions: head0 rows 0-63, head1 64-127)
    kTz = singles.tile([128, 2, N], FP8)           # slot0 = k8, slot1 = dk8
    vTz = singles.tile([128, 2, N], FP8)           # slot0 = slot1 = v8
    qTs = singles.tile([128, N], F32)              # Q^T channel-major staging
    qa = [singles.tile([128, NB, HD + 1], BF16, name=f"qa{h}") for h in range(HPC)]
    o_all = singles.tile([HD + 1, HPC, N], F32)    # O^T (+Z row) per head
    # f32r: feeds the proj matmul, which requires f32r-rounded producers
    mt = [singles.tile([128, CC, 512], F32R, name=f"mt{h}") for h in range(HPC)]



    make_identity(nc, id_sb)
    for h in range(HPC):
        nc.vector.memset(qa[h][:, :, HD:HD + 1], 1.0)

    # ---- input DMAs: small weights first, then x quarter-major ----
    nc.sync.dma_start(out=wq_sb, in_=wq_h.ap().rearrange("(cc p) m -> p cc m", p=128))
    nc.sync.dma_start(out=wk_sb, in_=wk_h.ap().rearrange("(cc p) m -> p cc m", p=128))
    nc.sync.dma_start(out=wv_sb, in_=wv_h.ap().rearrange("(cc p) m -> p cc m", p=128))
    x_view = x_h.ap().rearrange("(cc p) n -> p cc n", p=128)
    for q in range(4):
        for cc in range(CC):
            for piece in range(4):
                c0 = q * 1024 + piece * 256
                nc.sync.dma_start(
                    out=x_sb[:, cc, c0:c0 + 256], in_=x_view[:, cc, c0:c0 + 256]
                )
    nc.sync.dma_start(out=wp_sb, in_=wp_h.ap().rearrange("(cc p) m -> p cc m", p=128))
    nc.sync.dma_start(out=bias_sb, in_=bp_h.ap().to_broadcast((128, 512)))

    # ---- QKV phase ----
    def kv_group(w_sb, is_k, j8):
        kv_ps = spool.tile([128, 512], F32, tag="s", name="kv_ps")
        for cc in range(CC):
            nc.tensor.matmul(
                kv_ps,
                lhsT=r(w_sb[:, cc, :]),
                rhs=r(x_sb[:, cc, ts(j8, 512)]),
                start=(cc == 0),
                stop=(cc == CC - 1),
            )
        dz = kTz if is_k else vTz
        nc.vector.tensor_copy(out=dz[:, 0, ts(j8, 512)], in_=kv_ps)
        if is_k:  # slot1 = fp8 residual of k
            nc.vector.tensor_tensor(
                out=dz[:, 1, ts(j8, 512)], in0=kv_ps, in1=dz[:, 0, ts(j8, 512)], op=SUB
            )
        else:     # slot1 = copy of v8 (SBUF->SBUF, GPSIMD is otherwise idle)
            nc.gpsimd.tensor_copy(out=dz[:, 1, ts(j8, 512)], in_=dz[:, 0, ts(j8, 512)])

    def qT_group(j8):
        qt_ps = spool.tile([128, 512], F32, tag="s", name="qt_ps")
        for cc in range(CC):
            nc.tensor.matmul(
                qt_ps,
                lhsT=r(wq_sb[:, cc, :]),
                rhs=r(x_sb[:, cc, ts(j8, 512)]),
                start=(cc == 0),
                stop=(cc == CC - 1),
            )
        nc.vector.tensor_copy(out=qTs[:, ts(j8, 512)], in_=qt_ps)

    def qa_group(nb):
        # token-major q via PE transpose of qTs (f32r matmuls with <256
        # output columns are 4x slower, so q cannot be made token-major
        # directly at full speed)
        tq = spool.tile([128, 128], F32, tag="s", name="tq")
        nc.tensor.transpose(tq, qTs[:, ts(nb, 128)], id_sb)
        for h in range(HPC):
            nc.scalar.activation(
                out=qa[h][:, nb, 0:HD], in_=tq[:, ts(h, HD)], func=COPY
            )

    # ---- attention: flat pair stream -------------------------------------
    # Pair p = (chunk j = p//NB, m-block i = p%NB).  Per slot the PE runs
    # [S(p)h0, S(p)h1, O(p-LAG)h0, O(p-LAG)h1]; exp(p) as two consecutive
    # half-tile instructions on ONE engine (ACT or DVE, alternating by a
    # ~18:14 ratio).  Single-bank S tiles rotate through a 5-deep ring, so
    # the S -> exp -> S-reuse WAR loop (~1.6us) spreads over 2.5 pairs and
    # stays off the critical path.  Keeping both halves of a pair on one
    # engine matters: a cross-engine pair makes every S wait on the more
    # backlogged of two queues.  Chunk boundaries dissolve into the flat
    # stream (the O chain and its drain trail LAG pairs behind).
    e_ring = {}
    o_ring = {}
    st = {"o_done": None, "tt": None}

    def emit_s_exp(p):
        j, i = p // NB, p % NB
        e_t = epool.tile([128, 2, 512], BF16, tag="e", name="e_t")
        for h in range(HPC):
            s_t = spool.tile([128, 512], F32, tag="s", name="s_t")
            nc.tensor.matmul(
                s_t,
                lhsT=vTz[ts(h, HD), :, ts(i, 128)],
                rhs=kTz[ts(h, HD), :, ts(j, 512)],
                start=True,
                stop=True,
                perf_mode=DR,
            )
            # both halves of a pair stay on ONE engine (alternating pairs
            # ~17:15) — cross-engine pairs couple every S to the more
            # backlogged queue and measurably lose
            if (p * 17) % 32 < 17:
                nc.scalar.activation(
                    out=e_t[:, h, :], in_=s_t, func=EXP, scale=SCALE
                )
            else:
                nc.vector.tensor_scalar(
                    out=e_t.bitcast(I16)[:, h, :], in0=s_t,
                    scalar1=A8, scalar2=B8, op0=MULT, op1=ADD,
                )
        e_ring[p] = e_t

    def emit_drain(p):
        """Consumer-side work for pair p: O matmuls, then (trailing a
        finished chunk) the O->SBUF copies, transposes and normalization."""
        j, i = p // NB, p % NB
        if i == 0:
            o_ring[j] = [opool.tile([128, 512], F32, tag="o", name=f"o_ps{h}")
                         for h in range(HPC)]
        e_t = e_ring.pop(p)
        o_ps = o_ring[j]
        for h in range(HPC):
            nc.tensor.matmul(
                o_ps[h][0:HD + 1, :],
                lhsT=r(qa[h][:, i, :]),
                rhs=e_t[:, h, :],
                start=(i == 0),
                stop=(i == NB - 1),
            )
        if i == NB - 1:           # chunk j's O chain complete: copy out
            nc.scalar.activation(
                out=o_all[:, 0, ts(j, 512)], in_=o_ps[0][0:HD + 1, :],
                func=COPY,
            )
            nc.vector.tensor_copy(
                out=o_all[:, 1, ts(j, 512)], in_=o_ps[1][0:HD + 1, :]
            )
            st["o_done"] = j
            del o_ring[j]
        jd = st["o_done"]
        if jd is not None:
            # transposes of chunk jd trail the copies: h0 strips at slots
            # 2..5 (norm at 6), h1 reuses the same 4-slot tile at 7..10
            # (norm at 11) — one PSUM bank covers all 8 transposes
            if i in (2, 7):
                st["tt"] = spool.tile([128, 4, HD + 1], F32, tag="s",
                                      name="tt")
            if 2 <= i <= 5 or 7 <= i <= 10:
                h, c = (0, i - 2) if i <= 5 else (1, i - 7)
                nc.tensor.transpose(
                    st["tt"][:, c, :], o_all[:, h, ts(jd * 4 + c, 128)],
                    id_sb[0:HD + 1, 0:HD + 1],
                )
            elif i in (6, 11):
                h = 0 if i == 6 else 1
                t_t = st["tt"]
                rz4 = vpool.tile([128, 4, 1], F32, tag="rz", name="rz4")
                nc.vector.reciprocal(out=rz4, in_=t_t[:, :, HD:HD + 1])
                for c in range(4):
                    nc.vector.tensor_scalar_mul(
                        mt[h][:, c, jd::8], t_t[:, c, 0:HD], rz4[:, c, :],
                    )

    # ---- QKV interleaved with attention chunk 0 (fills the x-DMA wait) ----
    for q4 in range(4):
        j0 = 2 * q4
        kv_group(wv_sb, False, j0)
        kv_group(wv_sb, False, j0 + 1)
        qT_group(j0)
        for nb in range(8 * q4, 8 * q4 + 4):
            qa_group(nb)
        kv_group(wk_sb, True, j0)
        kv_group(wk_sb, True, j0 + 1)
        qT_group(j0 + 1)
        for nb in range(8 * q4 + 4, 8 * q4 + 8):
            qa_group(nb)
        # chunk-0 pairs whose vT m-blocks this quarter just produced
        for p in range(8 * q4, 8 * q4 + 8):
            emit_s_exp(p)
            if p >= LAG:
                emit_drain(p - LAG)

    # ---- flat stream: chunks 1..7 + trailing drains ----
    NP = NJ * NB
    for p in range(NB, NP + LAG):
        if p < NP:
            emit_s_exp(p)
        emit_drain(p - LAG)

    # last chunk's transposes + norms (no following chunk to carry them)
    for h in range(HPC):
        t_t = spool.tile([128, 4, HD + 1], F32, tag="s", name="tt")
        for c in range(4):
            nc.tensor.transpose(
                t_t[:, c, :], o_all[:, h, ts((NJ - 1) * 4 + c, 128)],
                id_sb[0:HD + 1, 0:HD + 1],
            )
        rz4 = vpool.tile([128, 4, 1], F32, tag="rz", name="rz4")
        nc.vector.reciprocal(out=rz4, in_=t_t[:, :, HD:HD + 1])
        for c in range(4):
            nc.vector.tensor_scalar_mul(
                mt[h][:, c, (NJ - 1)::8], t_t[:, c, 0:HD], rz4[:, c, :],
            )

    # ---- projection ----
    for h in range(HPC):
        for l in range(4):
            y_ps = spool.tile([128, 512], F32, tag="s", name="y_ps")
            for kk in range(CC):
                nc.tensor.matmul(
                    y_ps,
                    lhsT=r(mt[h][:, kk, ts(l, 128)]),
                    rhs=r(wp_sb[:, kk, :]),
                    start=(kk == 0),
                    stop=(kk == CC - 1),
                )
            y_sb = vpool.tile([128, 512], F32, tag="y", name="y_sb")
            nc.vector.tensor_add(out=y_sb, in0=y_ps, in1=bias_sb)
            nc.sync.dma_start(out=out_h.ap()[h, ts(l, 128), :], in_=y_sb)

    for pool in (vpool, epool, opool, spool, singles):
        pool.release()


_CACHE = {}


def _build():
    if "nc" not in _CACHE:
        nc = bacc.Bacc("TRN2", target_bir_lowering=False, debug=False)
        with tile.TileContext(nc) as tc:
            _emit(nc, tc)
        nc.compile()
        _CACHE["nc"] = nc
    return _CACHE["nc"]


def _shard(x, w_qkv, w_proj, b_proj):
    """Build the 8 per-core input maps from the full inputs."""
    wpT = np.ascontiguousarray(w_proj.T)
    bp = np.ascontiguousarray(b_proj.reshape(1, C))
    in_maps = []
    for core in range(N_CORES):
        b = core // 4
        h0 = HPC * (core % 4)
        r0 = h0 * HD
        in_maps.append({
            "x": np.ascontiguousarray(x[b].reshape(C, N)),
            "wq": np.ascontiguousarray(w_qkv[r0:r0 + 128, :].T),
            # NOTE: k left unscaled (hd**-0.5 folded into exp) so fp8
            # quantization sees full-range values
            "wk": np.ascontiguousarray(w_qkv[C + r0:C + r0 + 128, :].T),
            "wv": np.ascontiguousarray(w_qkv[2 * C + r0:2 * C + r0 + 128, :].T),
            "wp": wpT,
            "bp": bp,
        })
    return in_maps


def _gather(results):
    full = np.empty((B, C, N), dtype=np.float32)
    for core in range(N_CORES):
        b = core // 4
        h0 = HPC * (core % 4)
        y = results[core]["out"]  # [2, 512, 512]
        for hi in range(HPC):
            ch0 = (h0 + hi) * HD
            full[b, ch0:ch0 + HD] = y[hi].reshape(HD, N)
    return full.reshape(B, C, H, W)


def run(inputs, trace=False, **kw):
    nc = _build()
    in_maps = _shard(**inputs)
    res = bass_utils.run_bass_kernel_spmd(
        nc, in_maps, core_ids=list(range(N_CORES)), trace=trace, **kw
    )
    return _gather(res.results), res


def kernel(x, w_qkv, w_proj, b_proj):
    out, _ = run(dict(x=x, w_qkv=w_qkv, w_proj=w_proj, b_proj=b_proj))
    return out
